# revision 6
# baseline (speedup 1.0000x reference)
"""Trainium2 Bass kernel for an AttentionBlock (InstanceNorm + single-head
spatial self-attention + projection + residual).

Full-input contract: kernel(**inputs) takes the complete tensors and returns
the complete output. Internally shards across 8 NeuronCores: data-parallel
over batch (B=4 -> 4 pairs of cores), sequence-parallel over the N=4096 query
positions within each sample (2 cores per sample, 2048 queries each).

All 8 cores run the *same* program; the query-half assignment is done by
rotating the spatial columns of x host-side (attention and instance-norm
statistics are invariant under column permutation).

v3 design (transposed-scores / flash-style):
  - All matmuls f16 (hi/lo double for QKV projections, single f16 for the
    score matmuls; ~2.6e-3 rel_l2 end to end, tolerance 2e-2).
  - pass-1 ([q,k] layout): single f16 matmul per (q-tile, k-chunk), DVE
    row-max, PE-transposed into the bias row (partition 64) of the pass-2
    moving operand.
  - pass-2 ([k,q] layout): K=65 matmul (64 channels + bias row carrying
    -rowmax) produces shifted scores in PSUM; ScalarE exp writes f16
    attention blocks straight to SBUF in the layout attn@v consumes. No PE
    transposes of attention, no PSUM-evacuation copies.
  - attn@v: v is augmented with a ones column so softmax denominators fall
    out of the same accumulation; normalization is applied after the output
    projection (they commute).
  - Emission is software-pipelined (scores pair p, attn@v pair p-1, two
    pass-1 steps of the next chunk as PE filler) to keep the PE busy
    back-to-back so the HAM clock gate stays at 8/8 (2.4 GHz).
"""

import os
import sys
import numpy as np
from contextlib import ExitStack

for _p in ("/opt/trn_rl_repo", "/root/.axon_site/_ro/trn_rl_repo"):
    if os.path.isdir(_p) and _p not in sys.path:
        sys.path.append(_p)

from concourse import bass, bacc, tile, mybir, masks  # noqa: E402
from concourse.bass_utils import run_bass_kernel_spmd  # noqa: E402

F32 = mybir.dt.float32
F16 = mybir.dt.float16

B, C, H, W = 4, 64, 64, 64
N = H * W            # 4096 spatial positions (attention length)
HALF = N // 2        # queries per core
KT = 128             # pass-2 k-tile (partition dim of transposed scores)
NKT = N // KT        # 32 k-tiles
NPR = NKT // 2       # 16 k-tile pairs
QC = 512             # q-chunk (PSUM bank free dim)
NQC = HALF // QC     # 4 q-chunks per core
QT = 128             # pass-1 q-tile
KC = 512             # pass-1 k-chunk
NKC = N // KC        # 8
EPS = 1e-5
NCORES = 8


def build_nc():
    nc = bacc.Bacc("TRN2", target_bir_lowering=False, debug=False)

    x_d = nc.dram_tensor("x", [C, N], F32, kind="ExternalInput")
    wq_d = nc.dram_tensor("wq1", [C + 1, 2, C], F16, kind="ExternalInput")
    wk_d = nc.dram_tensor("wk1", [C + 1, 2, C], F16, kind="ExternalInput")
    wv_d = nc.dram_tensor("wv1", [C + 1, 2, C], F16, kind="ExternalInput")
    wo_d = nc.dram_tensor("wo16", [C, C], F16, kind="ExternalInput")
    bo_d = nc.dram_tensor("bo", [C, 1], F32, kind="ExternalInput")
    out_d = nc.dram_tensor("out", [C, HALF], F32, kind="ExternalOutput")

    with tile.TileContext(nc) as tc:
        _body(tc, x_d, wq_d, wk_d, wv_d, wo_d, bo_d, out_d)
    nc.compile()
    return nc


def _body(tc, x_d, wq_d, wk_d, wv_d, wo_d, bo_d, out_d):
    nc = tc.nc
    with ExitStack() as ctx:
        persist = ctx.enter_context(tc.tile_pool(name="persist", bufs=1))
        small = ctx.enter_context(tc.tile_pool(name="small", bufs=4))
        apool = ctx.enter_context(tc.tile_pool(name="apool", bufs=3))
        fpool = ctx.enter_context(tc.tile_pool(name="fpool", bufs=2))
        # PSUM: 2 (pass-1 etc) + 4 (scores, 2-bank pairs) + 2 (attn@v) banks
        p1p = ctx.enter_context(tc.tile_pool(name="p1p", bufs=2, space="PSUM"))
        scp = ctx.enter_context(tc.tile_pool(name="scp", bufs=2, space="PSUM"))
        avp = ctx.enter_context(tc.tile_pool(name="avp", bufs=2, space="PSUM"))

        # ---- inputs ----
        x_sb = persist.tile([C, N], F32)
        wq_sb = persist.tile([C + 1, 2, C], F16)
        nc.sync.dma_start(out=wq_sb, in_=wq_d.ap())
        wk_sb = persist.tile([C + 1, 2, C], F16)
        nc.scalar.dma_start(out=wk_sb, in_=wk_d.ap())
        wv_sb = persist.tile([C + 1, 2, C], F16)
        nc.sync.dma_start(out=wv_sb, in_=wv_d.ap())
        wo_sb = persist.tile([C, C], F16)
        nc.scalar.dma_start(out=wo_sb, in_=wo_d.ap())
        bo_sb = persist.tile([C, 1], F32)
        nc.sync.dma_start(out=bo_sb, in_=bo_d.ap())
        eps_t = persist.tile([C, 1], F32)
        nc.vector.memset(eps_t, EPS)
        ones16 = persist.tile([1, C], F16)
        nc.gpsimd.memset(ones16, 1.0)
        ident = persist.tile([QT, QT], F16)
        masks.make_identity(nc, ident)

        # x load split across queues; bn_stats per chunk as it arrives
        stats = persist.tile([C, NKC, nc.vector.BN_STATS_DIM], F32)
        for i in range(NKC):
            sl = slice(i * KC, (i + 1) * KC)
            eng = nc.sync if i % 2 == 0 else nc.scalar
            eng.dma_start(out=x_sb[:, sl], in_=x_d.ap()[:, sl])
            nc.vector.bn_stats(out=stats[:, i, :], in_=x_sb[:, sl])
        mv = persist.tile([C, nc.vector.BN_AGGR_DIM], F32)
        nc.vector.bn_aggr(out=mv, in_=stats)
        stdv = persist.tile([C, 1], F32)
        nc.scalar.activation(out=stdv, in_=mv[:, 1:2],
                             func=mybir.ActivationFunctionType.Sqrt,
                             bias=eps_t, scale=1.0)
        rstd = persist.tile([C, 1], F32)
        nc.vector.reciprocal(out=rstd, in_=stdv)
        nmr = persist.tile([C, 1], F32)
        nc.vector.tensor_mul(nmr, mv[:, 0:1], rstd)
        nc.vector.tensor_scalar_mul(nmr, nmr, -1.0)

        # xn (f32, for residual + lo-part), f16 hi/lo with bias-row for QKV
        xn = persist.tile([C, N], F32)
        xnh = persist.tile([C + 1, N], F16)
        xnl = persist.tile([C + 1, N], F16)
        nc.gpsimd.memset(xnh[C:C + 1, :], 1.0)
        nc.gpsimd.memset(xnl[C:C + 1, :], 0.0)
        for i in range(4):
            hl = slice(i * (N // 4), (i + 1) * (N // 4))
            nc.scalar.activation(out=xn[:, hl], in_=x_sb[:, hl],
                                 func=mybir.ActivationFunctionType.Identity,
                                 bias=nmr, scale=rstd)
            nc.vector.tensor_copy(xnh[0:C, hl], xn[:, hl])
            nc.vector.tensor_sub(xnl[0:C, hl], xn[:, hl], xnh[0:C, hl])
        # residual + output bias for our query half
        xnb = persist.tile([C, HALF], F32)
        nc.vector.tensor_scalar_add(xnb, xn[:, 0:HALF], bo_sb)

        # ---- QKV projections (f16 double: wh@xh + wh@xl) ----
        kst = persist.tile([C + 1, N], F16)      # rows 0:64 k, row 64 ones
        qrhs = persist.tile([C + 1, HALF], F16)  # rows 0:64 q*sqrt(C), row 64 -max
        nc.gpsimd.memset(kst[C:C + 1, :], 1.0)
        vst = persist.tile([KT, NKT, 66], F16)   # [kpos, ktile, 64 v + ones + pad]
        nc.gpsimd.memset(vst[:, :, 64:65], 1.0)
        nc.gpsimd.memset(vst[:, :, 65:66], 0.0)

        for i in range(NKC):
            sl = slice(i * KC, (i + 1) * KC)
            kp = scp.tile([KT, 2, KC], F32, tag="sc", name=f"kp{i}")
            nc.tensor.matmul(kp[0:C, 0, :], lhsT=wk_sb[:, 0, :], rhs=xnh[:, sl],
                             start=True, stop=False, skip_group_check=True)
            nc.tensor.matmul(kp[0:C, 0, :], lhsT=wk_sb[:, 0, :], rhs=xnl[:, sl],
                             start=False, stop=True, skip_group_check=True)
            if i % 2 == 0:
                nc.vector.tensor_copy(kst[0:C, sl], kp[0:C, 0, :])
            else:
                nc.scalar.copy(kst[0:C, sl], kp[0:C, 0, :])
        for i in range(NQC):
            sl = slice(i * QC, (i + 1) * QC)
            qp = scp.tile([KT, 2, KC], F32, tag="sc", name=f"qp{i}")
            nc.tensor.matmul(qp[0:C, 0, :], lhsT=wq_sb[:, 0, :], rhs=xnh[:, sl],
                             start=True, stop=False, skip_group_check=True)
            nc.tensor.matmul(qp[0:C, 0, :], lhsT=wq_sb[:, 0, :], rhs=xnl[:, sl],
                             start=False, stop=True, skip_group_check=True)
            if i % 2 == 0:
                nc.vector.tensor_copy(qrhs[0:C, sl], qp[0:C, 0, :])
            else:
                nc.scalar.copy(qrhs[0:C, sl], qp[0:C, 0, :])

        # ---- incremental pass-1 (row max of chunk c1's q-tiles) ----
        p1_state = {}

        def pass1_step(c1):
            """Emit one pass-1 step (one k-chunk matmul + DVE max) for chunk
            c1; every 8th step finalizes a q-tile's -max into qrhs row 64."""
            st = p1_state.setdefault(c1, {"step": 0})
            step = st["step"]
            if step >= 4 * NKC:
                return
            st["step"] = step + 1
            t4, ci = divmod(step, NKC)
            t = c1 * 4 + t4
            tq = slice(t * QT, (t + 1) * QT)
            if ci == 0:
                st["cm"] = small.tile([QT, NKC], F32, tag="cm", name=f"cm{t}")
            cm = st["cm"]
            cs = slice(ci * KC, (ci + 1) * KC)
            p1 = p1p.tile([QT, KC], F32, tag="p1", name=f"p1_{t}_{ci}")
            nc.tensor.matmul(p1, lhsT=qrhs[0:C, tq], rhs=kst[0:C, cs],
                             start=True, stop=True, skip_group_check=True)
            nc.vector.tensor_reduce(cm[:, ci:ci + 1], p1,
                                    axis=mybir.AxisListType.X,
                                    op=mybir.AluOpType.max)
            if ci == NKC - 1:
                nmT = small.tile([QT, C + 1], F16, tag="nmT", name=f"nmT{t}")
                nc.vector.tensor_reduce(nmT[:, C:C + 1], cm,
                                        axis=mybir.AxisListType.X,
                                        op=mybir.AluOpType.max, negate=True)
                tr = p1p.tile([C + 1, QT], F16, tag="p1", name=f"tr{t}")
                nc.tensor.transpose(tr, nmT, ident)
                nc.scalar.copy(qrhs[C:C + 1, tq], tr[C:C + 1, :])

        # v projection interleaved with chunk-0 pass-1
        for j in range(NKT):
            js = slice(j * KT, (j + 1) * KT)
            vp = p1p.tile([KT, C], F32, tag="p1", name=f"vp{j}")
            nc.tensor.matmul(vp, lhsT=xnh[:, js], rhs=wv_sb[:, 0, :],
                             start=True, stop=False, skip_group_check=True)
            nc.tensor.matmul(vp, lhsT=xnl[:, js], rhs=wv_sb[:, 0, :],
                             start=False, stop=True, skip_group_check=True)
            nc.scalar.copy(vst[:, j, 0:C], vp)
            pass1_step(0)

        # ---- main loop over q-chunks ----
        ao16 = persist.tile([C, HALF], F16)
        inv16 = persist.tile([1, HALF], F16)

        def emit_av(c, p, otp, ab):
            for h in range(2):
                j = 2 * p + h
                nc.tensor.matmul(otp, lhsT=vst[:, j, :], rhs=ab[:, h, :],
                                 start=(j == 0), stop=(j == NKT - 1),
                                 skip_group_check=True)

        for c in range(NQC):
            qs = slice(c * QC, (c + 1) * QC)
            otp = avp.tile([66, QC], F32, tag="av", name=f"otp{c}")
            prev_ab = None
            for p in range(NPR):
                sc = scp.tile([KT, 2, QC], F32, tag="sc", name=f"sc{c}_{p}")
                for h in range(2):
                    js = slice((2 * p + h) * KT, (2 * p + h + 1) * KT)
                    nc.tensor.matmul(sc[:, h, :], lhsT=kst[:, js],
                                     rhs=qrhs[:, qs],
                                     start=True, stop=True,
                                     skip_group_check=True)
                ab = apool.tile([KT, 2, QC], F16, tag="ab", name=f"ab{c}_{p}")
                nc.scalar.activation(out=ab.rearrange("p a b -> p (a b)"),
                                     in_=sc.rearrange("p a b -> p (a b)"),
                                     func=mybir.ActivationFunctionType.Exp,
                                     bias=0.0, scale=1.0)
                if prev_ab is not None:
                    emit_av(c, p - 1, otp, prev_ab)
                prev_ab = ab
                if c + 1 < NQC:
                    pass1_step(c + 1)
                    pass1_step(c + 1)
            emit_av(c, NPR - 1, otp, prev_ab)

            # epilogue for chunk c
            nc.scalar.copy(ao16[:, qs], otp[0:C, :])
            with nc.allow_low_precision(reason="1/sum fits f16; rel tol 2e-2"):
                nc.vector.reciprocal(out=inv16[:, qs], in_=otp[C:C + 1, :])
            fx = p1p.tile([KT, QC], F32, tag="p1", name=f"fx{c}")
            nc.tensor.matmul(fx[0:C, :], lhsT=wo_sb, rhs=ao16[:, qs],
                             start=True, stop=True, skip_group_check=True)
            fx2 = p1p.tile([KT, QC], F32, tag="p1", name=f"fx2{c}")
            nc.tensor.matmul(fx2[0:C, :], lhsT=ones16, rhs=inv16[:, qs],
                             start=True, stop=True, skip_group_check=True)
            ibs = fpool.tile([C, QC], F32, tag="ibs", name=f"ibs{c}")
            nc.scalar.copy(ibs, fx2[0:C, :])
            fin = fpool.tile([C, QC], F32, tag="fin", name=f"fin{c}")
            nc.vector.tensor_mul(fin, fx[0:C, :], ibs)
            nc.vector.tensor_add(fin, fin, xnb[:, qs])
            eng = nc.sync if c % 2 == 0 else nc.scalar
            eng.dma_start(out=out_d.ap()[:, qs], in_=fin)


def prep_inputs(x, w_qkv, b_qkv, w_out, b_out):
    """Host-side slicing/packing into per-core input maps."""
    x = np.asarray(x, dtype=np.float32).reshape(B, C, N)
    w_qkv = np.asarray(w_qkv, dtype=np.float32)
    b_qkv = np.asarray(b_qkv, dtype=np.float32)
    w_out = np.asarray(w_out, dtype=np.float32)
    b_out = np.asarray(b_out, dtype=np.float32)

    s = float(C) ** 0.5  # reference multiplies scores by sqrt(C)
    wq1 = np.concatenate([s * w_qkv[0:C].T, s * b_qkv[None, 0:C]], axis=0)
    wk1 = np.concatenate([w_qkv[C:2 * C].T, b_qkv[None, C:2 * C]], axis=0)
    wv1 = np.concatenate([w_qkv[2 * C:3 * C].T, b_qkv[None, 2 * C:3 * C]], axis=0)

    def hilo16(w):  # [65, 64] -> [65, 2, 64] f16 (hi, lo), hi+lo ~== w
        hi = w.astype(np.float16)
        lo = (w - hi.astype(np.float32)).astype(np.float16)
        return np.ascontiguousarray(np.stack([hi, lo], axis=1))

    wq1 = hilo16(np.ascontiguousarray(wq1))
    wk1 = hilo16(np.ascontiguousarray(wk1))
    wv1 = hilo16(np.ascontiguousarray(wv1))
    wo16 = np.ascontiguousarray(w_out.T).astype(np.float16)
    bo = np.ascontiguousarray(b_out[:, None])

    in_maps = []
    for j in range(NCORES):
        b, h = divmod(j, 2)
        xs = x[b]
        if h == 1:
            xs = np.concatenate([xs[:, HALF:], xs[:, :HALF]], axis=1)
        in_maps.append({
            "x": np.ascontiguousarray(xs),
            "wq1": wq1,
            "wk1": wk1,
            "wv1": wv1,
            "wo16": wo16,
            "bo": bo,
        })
    return in_maps


def gather_output(results):
    out = np.empty((B, C, N), dtype=np.float32)
    for j in range(NCORES):
        b, h = divmod(j, 2)
        out[b][:, h * HALF:(h + 1) * HALF] = results[j]["out"]
    return out.reshape(B, C, H, W)


_NC_CACHE = {}


def get_nc():
    key = "v3"
    if key not in _NC_CACHE:
        _NC_CACHE[key] = build_nc()
    return _NC_CACHE[key]


def kernel(x, w_qkv, b_qkv, w_out, b_out):
    nc = get_nc()
    in_maps = prep_inputs(x, w_qkv, b_qkv, w_out, b_out)
    res = run_bass_kernel_spmd(nc, in_maps, list(range(NCORES)))
    return gather_output(res.results)


# revision 9
# speedup vs baseline: 1.0818x; 1.0818x over previous
"""Trainium2 Bass kernel for an AttentionBlock (InstanceNorm + single-head
spatial self-attention + projection + residual).

Full-input contract: kernel(**inputs) takes the complete tensors and returns
the complete output. Internally shards across 8 NeuronCores: data-parallel
over batch (B=4 -> 4 pairs of cores), sequence-parallel over the N=4096 query
positions within each sample (2 cores per sample, 2048 queries each).

All 8 cores run the *same* program; the query-half assignment is done by
rotating the spatial columns of x host-side (attention and instance-norm
statistics are invariant under column permutation).

v3 design (transposed-scores / flash-style):
  - All matmuls f16 (hi/lo double for QKV projections, single f16 for the
    score matmuls; ~2.6e-3 rel_l2 end to end, tolerance 2e-2).
  - pass-1 ([q,k] layout): single f16 matmul per (q-tile, k-chunk), DVE
    row-max, PE-transposed into the bias row (partition 64) of the pass-2
    moving operand.
  - pass-2 ([k,q] layout): K=65 matmul (64 channels + bias row carrying
    -rowmax) produces shifted scores in PSUM; ScalarE exp writes f16
    attention blocks straight to SBUF in the layout attn@v consumes. No PE
    transposes of attention, no PSUM-evacuation copies.
  - attn@v: v is augmented with a ones column so softmax denominators fall
    out of the same accumulation; normalization is applied after the output
    projection (they commute).
  - Emission is software-pipelined (scores pair p, attn@v pair p-1, two
    pass-1 steps of the next chunk as PE filler) to keep the PE busy
    back-to-back so the HAM clock gate stays at 8/8 (2.4 GHz).
"""

import os
import sys
import numpy as np
from contextlib import ExitStack

for _p in ("/opt/trn_rl_repo", "/root/.axon_site/_ro/trn_rl_repo"):
    if os.path.isdir(_p) and _p not in sys.path:
        sys.path.append(_p)

from concourse import bass, bacc, tile, mybir, masks  # noqa: E402
from concourse.bass_utils import run_bass_kernel_spmd  # noqa: E402

F32 = mybir.dt.float32
F16 = mybir.dt.float16

B, C, H, W = 4, 64, 64, 64
N = H * W            # 4096 spatial positions (attention length)
HALF = N // 2        # queries per core
KT = 128             # pass-2 k-tile (partition dim of transposed scores)
NKT = N // KT        # 32 k-tiles
NPR = NKT // 2       # 16 k-tile pairs
QC = 512             # q-chunk (PSUM bank free dim)
NQC = HALF // QC     # 4 q-chunks per core
QT = 128             # pass-1 q-tile
KC = 512             # pass-1 k-chunk
NKC = N // KC        # 8
EPS = 1e-5
NCORES = 8


def build_nc():
    nc = bacc.Bacc("TRN2", target_bir_lowering=False, debug=False)

    x_d = nc.dram_tensor("x", [C, N], F32, kind="ExternalInput")
    wq_d = nc.dram_tensor("wq1", [C + 1, 2, C], F16, kind="ExternalInput")
    wk_d = nc.dram_tensor("wk1", [C + 1, 2, C], F16, kind="ExternalInput")
    wv_d = nc.dram_tensor("wv1", [C + 1, 2, C], F16, kind="ExternalInput")
    wo_d = nc.dram_tensor("wo16", [C, C], F16, kind="ExternalInput")
    bo_d = nc.dram_tensor("bo", [C, 1], F32, kind="ExternalInput")
    out_d = nc.dram_tensor("out", [C, HALF], F32, kind="ExternalOutput")

    with tile.TileContext(nc) as tc:
        _body(tc, x_d, wq_d, wk_d, wv_d, wo_d, bo_d, out_d)
    nc.compile()
    return nc


def _body(tc, x_d, wq_d, wk_d, wv_d, wo_d, bo_d, out_d):
    nc = tc.nc
    with ExitStack() as ctx:
        persist = ctx.enter_context(tc.tile_pool(name="persist", bufs=1))
        small = ctx.enter_context(tc.tile_pool(name="small", bufs=4))
        apool = ctx.enter_context(tc.tile_pool(name="apool", bufs=6))
        fpool = ctx.enter_context(tc.tile_pool(name="fpool", bufs=2))
        # PSUM: 2 (pass-1 etc) + 4 (scores, 2-bank pairs) + 2 (attn@v) banks
        p1p = ctx.enter_context(tc.tile_pool(name="p1p", bufs=2, space="PSUM"))
        scp = ctx.enter_context(tc.tile_pool(name="scp", bufs=2, space="PSUM"))
        avp = ctx.enter_context(tc.tile_pool(name="avp", bufs=2, space="PSUM"))

        # ---- inputs ----
        x_sb = persist.tile([C, N], F32)
        wq_sb = persist.tile([C + 1, 2, C], F16)
        nc.sync.dma_start(out=wq_sb, in_=wq_d.ap())
        wk_sb = persist.tile([C + 1, 2, C], F16)
        nc.scalar.dma_start(out=wk_sb, in_=wk_d.ap())
        wv_sb = persist.tile([C + 1, 2, C], F16)
        nc.sync.dma_start(out=wv_sb, in_=wv_d.ap())
        wo_sb = persist.tile([C, C], F16)
        nc.scalar.dma_start(out=wo_sb, in_=wo_d.ap())
        bo_sb = persist.tile([C, 1], F32)
        nc.sync.dma_start(out=bo_sb, in_=bo_d.ap())
        eps_t = persist.tile([C, 1], F32)
        nc.vector.memset(eps_t, EPS)
        ones16 = persist.tile([1, C], F16)
        nc.gpsimd.memset(ones16, 1.0)
        ident = persist.tile([QT, QT], F16)
        masks.make_identity(nc, ident)
        dmy = persist.tile([QT, KC], F16)
        nc.gpsimd.memset(dmy, 0.25)

        def warm_pe(tag, n):
            """Back-to-back dummy matmuls (no readers -> no stalls) that keep
            the PE busy through a full HAM window so the clock gate opens."""
            for w in range(n):
                wp = p1p.tile([QT, KC], F32, tag="p1", name=f"wp{tag}_{w}")
                nc.tensor.matmul(wp, lhsT=ident, rhs=dmy,
                                 start=True, stop=True, skip_group_check=True)

        # x load split across queues; bn_stats per chunk as it arrives
        stats = persist.tile([C, NKC, nc.vector.BN_STATS_DIM], F32)
        warm_pe("h", 12)
        for i in range(NKC):
            sl = slice(i * KC, (i + 1) * KC)
            eng = nc.sync if i % 2 == 0 else nc.scalar
            eng.dma_start(out=x_sb[:, sl], in_=x_d.ap()[:, sl])
            nc.vector.bn_stats(out=stats[:, i, :], in_=x_sb[:, sl])
        mv = persist.tile([C, nc.vector.BN_AGGR_DIM], F32)
        nc.vector.bn_aggr(out=mv, in_=stats)
        stdv = persist.tile([C, 1], F32)
        nc.scalar.activation(out=stdv, in_=mv[:, 1:2],
                             func=mybir.ActivationFunctionType.Sqrt,
                             bias=eps_t, scale=1.0)
        rstd = persist.tile([C, 1], F32)
        nc.vector.reciprocal(out=rstd, in_=stdv)
        nmr = persist.tile([C, 1], F32)
        nc.vector.tensor_mul(nmr, mv[:, 0:1], rstd)
        nc.vector.tensor_scalar_mul(nmr, nmr, -1.0)

        # xn (f32, for residual + lo-part), f16 hi/lo with bias-row for QKV
        xn = persist.tile([C, N], F32)
        xnh = persist.tile([C + 1, N], F16)
        xnl = persist.tile([C + 1, N], F16)
        nc.gpsimd.memset(xnh[C:C + 1, :], 1.0)
        nc.gpsimd.memset(xnl[C:C + 1, :], 0.0)
        for i in range(4):
            hl = slice(i * (N // 4), (i + 1) * (N // 4))
            nc.scalar.activation(out=xn[:, hl], in_=x_sb[:, hl],
                                 func=mybir.ActivationFunctionType.Identity,
                                 bias=nmr, scale=rstd)
            nc.vector.tensor_copy(xnh[0:C, hl], xn[:, hl])
            nc.vector.tensor_sub(xnl[0:C, hl], xn[:, hl], xnh[0:C, hl])
        # residual + output bias for our query half
        xnb = persist.tile([C, HALF], F32)
        nc.vector.tensor_scalar_add(xnb, xn[:, 0:HALF], bo_sb)

        # ---- QKV projections (f16 double: wh@xh + wh@xl) ----
        kst = persist.tile([C + 1, N], F16)      # rows 0:64 k, row 64 ones
        qrhs = persist.tile([C + 1, HALF], F16)  # rows 0:64 q*sqrt(C), row 64 -max
        nc.gpsimd.memset(kst[C:C + 1, :], 1.0)
        vst = persist.tile([KT, NKT, 66], F16)   # [kpos, ktile, 64 v + ones + pad]
        nc.gpsimd.memset(vst[:, :, 64:65], 1.0)
        nc.gpsimd.memset(vst[:, :, 65:66], 0.0)

        for i in range(NKC):
            sl = slice(i * KC, (i + 1) * KC)
            kp = scp.tile([KT, 2, KC], F32, tag="sc", name=f"kp{i}")
            nc.tensor.matmul(kp[0:C, 0, :], lhsT=wk_sb[:, 0, :], rhs=xnh[:, sl],
                             start=True, stop=False, skip_group_check=True)
            nc.tensor.matmul(kp[0:C, 0, :], lhsT=wk_sb[:, 0, :], rhs=xnl[:, sl],
                             start=False, stop=True, skip_group_check=True)
            if i % 2 == 0:
                nc.vector.tensor_copy(kst[0:C, sl], kp[0:C, 0, :])
            else:
                nc.scalar.copy(kst[0:C, sl], kp[0:C, 0, :])
        for i in range(NQC):
            sl = slice(i * QC, (i + 1) * QC)
            qp = scp.tile([KT, 2, KC], F32, tag="sc", name=f"qp{i}")
            nc.tensor.matmul(qp[0:C, 0, :], lhsT=wq_sb[:, 0, :], rhs=xnh[:, sl],
                             start=True, stop=False, skip_group_check=True)
            nc.tensor.matmul(qp[0:C, 0, :], lhsT=wq_sb[:, 0, :], rhs=xnl[:, sl],
                             start=False, stop=True, skip_group_check=True)
            if i % 2 == 0:
                nc.vector.tensor_copy(qrhs[0:C, sl], qp[0:C, 0, :])
            else:
                nc.scalar.copy(qrhs[0:C, sl], qp[0:C, 0, :])

        # ---- incremental pass-1 (row max of chunk c1's q-tiles) ----
        p1_state = {}

        def pass1_step(c1):
            """Emit one pass-1 step (one k-chunk matmul + DVE max) for chunk
            c1; every 8th step finalizes a q-tile's -max into qrhs row 64."""
            st = p1_state.setdefault(c1, {"step": 0})
            step = st["step"]
            if step >= 4 * NKC:
                return
            st["step"] = step + 1
            t4, ci = divmod(step, NKC)
            t = c1 * 4 + t4
            tq = slice(t * QT, (t + 1) * QT)
            if ci == 0:
                st["cm"] = small.tile([QT, NKC], F32, tag="cm", name=f"cm{t}")
            cm = st["cm"]
            cs = slice(ci * KC, (ci + 1) * KC)
            p1 = p1p.tile([QT, KC], F32, tag="p1", name=f"p1_{t}_{ci}")
            nc.tensor.matmul(p1, lhsT=qrhs[0:C, tq], rhs=kst[0:C, cs],
                             start=True, stop=True, skip_group_check=True)
            nc.vector.tensor_reduce(cm[:, ci:ci + 1], p1,
                                    axis=mybir.AxisListType.X,
                                    op=mybir.AluOpType.max)
            if ci == NKC - 1:
                nmT = small.tile([QT, C + 1], F16, tag="nmT", name=f"nmT{t}")
                nc.vector.tensor_reduce(nmT[:, C:C + 1], cm,
                                        axis=mybir.AxisListType.X,
                                        op=mybir.AluOpType.max, negate=True)
                tr = p1p.tile([C + 1, QT], F16, tag="p1", name=f"tr{t}")
                nc.tensor.transpose(tr, nmT, ident)
                nc.scalar.copy(qrhs[C:C + 1, tq], tr[C:C + 1, :])

        # v projection interleaved with chunk-0 pass-1
        for j in range(NKT):
            js = slice(j * KT, (j + 1) * KT)
            vp = p1p.tile([KT, C], F32, tag="p1", name=f"vp{j}")
            nc.tensor.matmul(vp, lhsT=xnh[:, js], rhs=wv_sb[:, 0, :],
                             start=True, stop=False, skip_group_check=True)
            nc.tensor.matmul(vp, lhsT=xnl[:, js], rhs=wv_sb[:, 0, :],
                             start=False, stop=True, skip_group_check=True)
            nc.scalar.copy(vst[:, j, 0:C], vp)
            pass1_step(0)

        # ---- main loop over q-chunks ----
        # attn@v matmuls trail the score/exp conveyor by AV_LAG pairs so they
        # never head-of-line block the in-order PE queue on a fresh exp; the
        # per-chunk epilogue rides the FIFO too (lands early in the next
        # chunk), which keeps the chunk boundary free of PE stalls.
        ao16 = persist.tile([C, HALF], F16)
        inv16 = persist.tile([1, HALF], F16)
        AV_LAG = 4
        av_fifo = []

        def emit_av(c, p, otp, ab):
            for h in range(2):
                j = 2 * p + h
                nc.tensor.matmul(otp, lhsT=vst[:, j, :], rhs=ab[:, h, :],
                                 start=(j == 0), stop=(j == NKT - 1),
                                 skip_group_check=True)

        def emit_epilogue(c, otp):
            qs = slice(c * QC, (c + 1) * QC)
            nc.scalar.copy(ao16[:, qs], otp[0:C, :])
            with nc.allow_low_precision(reason="1/sum fits f16; rel tol 2e-2"):
                nc.vector.reciprocal(out=inv16[:, qs], in_=otp[C:C + 1, :])
            fx = p1p.tile([KT, QC], F32, tag="p1", name=f"fx{c}")
            nc.tensor.matmul(fx[0:C, :], lhsT=wo_sb, rhs=ao16[:, qs],
                             start=True, stop=True, skip_group_check=True)
            fx2 = p1p.tile([KT, QC], F32, tag="p1", name=f"fx2{c}")
            nc.tensor.matmul(fx2[0:C, :], lhsT=ones16, rhs=inv16[:, qs],
                             start=True, stop=True, skip_group_check=True)
            ibs = fpool.tile([C, QC], F32, tag="ibs", name=f"ibs{c}")
            nc.scalar.copy(ibs, fx2[0:C, :])
            fin = fpool.tile([C, QC], F32, tag="fin", name=f"fin{c}")
            nc.vector.tensor_mul(fin, fx[0:C, :], ibs)
            nc.vector.tensor_add(fin, fin, xnb[:, qs])
            eng = nc.sync if c % 2 == 0 else nc.scalar
            eng.dma_start(out=out_d.ap()[:, qs], in_=fin)

        def pop_av():
            c0, p0, otp0, ab0 = av_fifo.pop(0)
            emit_av(c0, p0, otp0, ab0)
            if p0 == NPR - 1:
                emit_epilogue(c0, otp0)

        for c in range(NQC):
            qs = slice(c * QC, (c + 1) * QC)
            otp = avp.tile([66, QC], F32, tag="av", name=f"otp{c}")
            for p in range(NPR):
                sc = scp.tile([KT, 2, QC], F32, tag="sc", name=f"sc{c}_{p}")
                for h in range(2):
                    js = slice((2 * p + h) * KT, (2 * p + h + 1) * KT)
                    nc.tensor.matmul(sc[:, h, :], lhsT=kst[:, js],
                                     rhs=qrhs[:, qs],
                                     start=True, stop=True,
                                     skip_group_check=True)
                ab = apool.tile([KT, 2, QC], F16, tag="ab", name=f"ab{c}_{p}")
                nc.scalar.activation(out=ab.rearrange("p a b -> p (a b)"),
                                     in_=sc.rearrange("p a b -> p (a b)"),
                                     func=mybir.ActivationFunctionType.Exp,
                                     bias=0.0, scale=1.0)
                av_fifo.append((c, p, otp, ab))
                if len(av_fifo) > AV_LAG:
                    pop_av()
                if c + 1 < NQC:
                    pass1_step(c + 1)
                    pass1_step(c + 1)
                else:
                    # keep the PE dense through the last chunk
                    wd = p1p.tile([QT, KC], F32, tag="p1", name=f"wd{p}")
                    nc.tensor.matmul(wd, lhsT=ident, rhs=dmy,
                                     start=True, stop=True,
                                     skip_group_check=True)
        while av_fifo:
            pop_av()


def prep_inputs(x, w_qkv, b_qkv, w_out, b_out):
    """Host-side slicing/packing into per-core input maps."""
    x = np.asarray(x, dtype=np.float32).reshape(B, C, N)
    w_qkv = np.asarray(w_qkv, dtype=np.float32)
    b_qkv = np.asarray(b_qkv, dtype=np.float32)
    w_out = np.asarray(w_out, dtype=np.float32)
    b_out = np.asarray(b_out, dtype=np.float32)

    s = float(C) ** 0.5  # reference multiplies scores by sqrt(C)
    wq1 = np.concatenate([s * w_qkv[0:C].T, s * b_qkv[None, 0:C]], axis=0)
    wk1 = np.concatenate([w_qkv[C:2 * C].T, b_qkv[None, C:2 * C]], axis=0)
    wv1 = np.concatenate([w_qkv[2 * C:3 * C].T, b_qkv[None, 2 * C:3 * C]], axis=0)

    def hilo16(w):  # [65, 64] -> [65, 2, 64] f16 (hi, lo), hi+lo ~== w
        hi = w.astype(np.float16)
        lo = (w - hi.astype(np.float32)).astype(np.float16)
        return np.ascontiguousarray(np.stack([hi, lo], axis=1))

    wq1 = hilo16(np.ascontiguousarray(wq1))
    wk1 = hilo16(np.ascontiguousarray(wk1))
    wv1 = hilo16(np.ascontiguousarray(wv1))
    wo16 = np.ascontiguousarray(w_out.T).astype(np.float16)
    bo = np.ascontiguousarray(b_out[:, None])

    in_maps = []
    for j in range(NCORES):
        b, h = divmod(j, 2)
        xs = x[b]
        if h == 1:
            xs = np.concatenate([xs[:, HALF:], xs[:, :HALF]], axis=1)
        in_maps.append({
            "x": np.ascontiguousarray(xs),
            "wq1": wq1,
            "wk1": wk1,
            "wv1": wv1,
            "wo16": wo16,
            "bo": bo,
        })
    return in_maps


def gather_output(results):
    out = np.empty((B, C, N), dtype=np.float32)
    for j in range(NCORES):
        b, h = divmod(j, 2)
        out[b][:, h * HALF:(h + 1) * HALF] = results[j]["out"]
    return out.reshape(B, C, H, W)


_NC_CACHE = {}


def get_nc():
    key = "v3"
    if key not in _NC_CACHE:
        _NC_CACHE[key] = build_nc()
    return _NC_CACHE[key]


def kernel(x, w_qkv, b_qkv, w_out, b_out):
    nc = get_nc()
    in_maps = prep_inputs(x, w_qkv, b_qkv, w_out, b_out)
    res = run_bass_kernel_spmd(nc, in_maps, list(range(NCORES)))
    return gather_output(res.results)
